# revision 6
# baseline (speedup 1.0000x reference)
"""Chamfer loss kernel for Trainium2 (8 NeuronCores, SPMD data-parallel over batch).

Math: for each batch b, d2[n, m] = |p_n|^2 + |g_m|^2 - 2 p_n.g_m is produced
directly by a K=5 augmented matmul on the PE:
  lhsT = [-2*px, -2*py, -2*pz, 1, |p|^2]  (5 x N, stationary)
  rhs  = [ gx,    gy,   gz,  |g|^2, 1 ]   (5 x M, moving)
Row mins (over gt) and column mins (over pred) of d2 are reduced on-device;
sqrt and the means run on host (min/max commute with the monotonic sqrt/clamp).

Each core handles 4 of the 32 batches. No collectives; host combines scalars.
"""

import sys

for _p in ("/opt/trn_rl_repo",):
    if _p not in sys.path:
        sys.path.insert(0, _p)

from contextlib import ExitStack
from functools import lru_cache

import numpy as np

import concourse.bass as bass
import concourse.tile as tile
from concourse import bacc, masks, mybir
from concourse.bass_utils import run_bass_kernel_spmd

F32 = mybir.dt.float32
MIN = mybir.AluOpType.min

B, N, M = 32, 4096, 4096
NCORES = 8
BPC = B // NCORES  # batches per core
K = 5              # augmented contraction dim
PCH = 128          # pred chunk size (PE partitions)
NP = N // PCH      # 32 pred chunks
FD = 1024          # psum tile free size (2 PSUM banks)
NG = M // FD       # 4 gt chunks
MMN = 512          # fp32 matmul max moving free dim
BIG = 3.0e38


def _build_program(gpsimd_tt_every: int = 0):
    """Build + compile the per-core Bass program.

    gpsimd_tt_every: every k-th colstate update runs on GpSimd instead of DVE
    (0 = all on DVE) to balance the two reduction-capable engines.
    """
    nc = bacc.Bacc(
        "TRN2", target_bir_lowering=False, debug=False, num_devices=NCORES
    )
    lhs = nc.dram_tensor("lhs", [BPC * K, N], F32, kind="ExternalInput").ap()
    rhs = nc.dram_tensor("rhs", [BPC * K, M], F32, kind="ExternalInput").ap()
    rowmin = nc.dram_tensor("rowmin", [BPC * PCH, NP], F32, kind="ExternalOutput").ap()
    colmin = nc.dram_tensor(
        "colmin", [BPC * PCH, M // PCH], F32, kind="ExternalOutput"
    ).ap()

    with tile.TileContext(nc) as tc, ExitStack() as ctx:
        const_pool = ctx.enter_context(tc.tile_pool(name="const", bufs=1))
        ident = const_pool.tile([PCH, PCH], F32)
        masks.make_identity(nc, ident[:])
        inf_t = const_pool.tile([PCH, FD], F32)
        nc.gpsimd.memset(inf_t[:], BIG)

        lr_pool = ctx.enter_context(tc.tile_pool(name="lr", bufs=2))
        col_pool = ctx.enter_context(tc.tile_pool(name="col", bufs=NG + 1))
        acc_pool = ctx.enter_context(tc.tile_pool(name="acc", bufs=2))
        scr_pool = ctx.enter_context(tc.tile_pool(name="scr", bufs=8))
        psum_pool = ctx.enter_context(tc.tile_pool(name="psum", bufs=3, space="PSUM"))
        psumt_pool = ctx.enter_context(tc.tile_pool(name="psumt", bufs=2, space="PSUM"))

        for i in range(BPC):
            L = lr_pool.tile([K, N], F32, tag="L")
            nc.sync.dma_start(L[:], lhs[K * i : K * (i + 1), :])
            R = lr_pool.tile([K, M], F32, tag="R")
            nc.sync.dma_start(R[:], rhs[K * i : K * (i + 1), :])

            colstate = [
                col_pool.tile([PCH, FD], F32, tag="cs", name=f"cs_{i}_{g}")
                for g in range(NG)
            ]
            rowacc = acc_pool.tile([PCH, NP], F32, tag="rowacc")
            colfin = acc_pool.tile([PCH, M // PCH], F32, tag="colfin")

            for p in range(NP):
                rowpart = scr_pool.tile(
                    [PCH, NG], F32, tag="rowpart", name=f"rp_{i}_{p}"
                )
                for g in range(NG):
                    ps = psum_pool.tile([PCH, FD], F32, tag="ps")
                    for s in range(FD // MMN):
                        nc.tensor.matmul(
                            ps[:, MMN * s : MMN * (s + 1)],
                            lhsT=L[:, PCH * p : PCH * (p + 1)],
                            rhs=R[:, FD * g + MMN * s : FD * g + MMN * (s + 1)],
                            start=True,
                            stop=True,
                        )
                    # row (pred-point) partial min over this gt chunk
                    nc.vector.tensor_reduce(
                        out=rowpart[:, g : g + 1],
                        in_=ps[:],
                        axis=mybir.AxisListType.X,
                        op=MIN,
                    )
                    # colstate accumulation (min over pred chunks).
                    src0 = inf_t[:] if p == 0 else colstate[g][:]
                    nc.vector.tensor_tensor(
                        out=colstate[g][:], in0=src0, in1=ps[:], op=MIN
                    )
                # fold the NG partials into the final row min for chunk p
                nc.vector.tensor_reduce(
                    out=rowacc[:, p : p + 1],
                    in_=rowpart[:],
                    axis=mybir.AxisListType.X,
                    op=MIN,
                )

            # Column mins: partition-reduce each colstate tile via PE
            # transpose + free-axis min.
            for g in range(NG):
                for blk in range(FD // PCH):
                    pt = psumt_pool.tile([PCH, PCH], F32, tag="pt")
                    nc.tensor.transpose(
                        pt[:], colstate[g][:, PCH * blk : PCH * (blk + 1)], ident[:]
                    )
                    j = g * (FD // PCH) + blk
                    nc.vector.tensor_reduce(
                        out=colfin[:, j : j + 1],
                        in_=pt[:],
                        axis=mybir.AxisListType.X,
                        op=MIN,
                    )

            nc.sync.dma_start(rowmin[PCH * i : PCH * (i + 1), :], rowacc[:])
            nc.sync.dma_start(colmin[PCH * i : PCH * (i + 1), :], colfin[:])

    nc.compile()
    return nc


@lru_cache(maxsize=1)
def _get_program():
    return _build_program()


def _make_inputs(pred, gt):
    """Host-side packing of the augmented [K, N] operands, per core."""
    pred = np.ascontiguousarray(pred, dtype=np.float32)
    gt = np.ascontiguousarray(gt, dtype=np.float32)
    p2 = np.einsum("bnd,bnd->bn", pred, pred)
    g2 = np.einsum("bmd,bmd->bm", gt, gt)
    lhs = np.empty((B, K, N), np.float32)
    lhs[:, 0:3] = -2.0 * pred.transpose(0, 2, 1)
    lhs[:, 3] = 1.0
    lhs[:, 4] = p2
    rhs = np.empty((B, K, M), np.float32)
    rhs[:, 0:3] = gt.transpose(0, 2, 1)
    rhs[:, 3] = g2
    rhs[:, 4] = 1.0
    in_maps = []
    for c in range(NCORES):
        sl = slice(c * BPC, (c + 1) * BPC)
        in_maps.append(
            {
                "lhs": np.ascontiguousarray(lhs[sl].reshape(BPC * K, N)),
                "rhs": np.ascontiguousarray(rhs[sl].reshape(BPC * K, M)),
            }
        )
    return in_maps


def _finish(results):
    rowmins = np.stack([r["rowmin"] for r in results])  # [8, BPC*128, 32]
    colmins = np.stack([r["colmin"] for r in results])
    ch2 = np.sqrt(np.maximum(rowmins.astype(np.float64), 1e-12)).mean()
    ch1 = np.sqrt(np.maximum(colmins.astype(np.float64), 1e-12)).mean()
    return np.asarray(ch1 + ch2, dtype=np.float32)


def kernel(pred, gt):
    nc = _get_program()
    in_maps = _make_inputs(pred, gt)
    res = run_bass_kernel_spmd(nc, in_maps, list(range(NCORES)))
    return _finish(res.results)


if __name__ == "__main__":
    rng = np.random.default_rng(0)
    pred = rng.standard_normal((B, N, 3), dtype=np.float32)
    gt = rng.standard_normal((B, M, 3), dtype=np.float32)
    print(kernel(pred, gt))


# revision 7
# speedup vs baseline: 1.8031x; 1.8031x over previous
"""Chamfer loss kernel for Trainium2 (8 NeuronCores, SPMD data-parallel over batch).

Math: for each batch b, d2[n, m] = |p_n|^2 + |g_m|^2 - 2 p_n.g_m is produced
directly by an augmented matmul on the PE. To run the PE at bf16 rate (1
cycle/row instead of fp32's 4) without losing fp32 accuracy, every fp32
operand is split into three bf16 terms (h + m + l); retaining the product
pairs hh, hm, mh, hl, lh, mm reproduces each fp32 product to ~2^-27 rel.
With 3 coords x 6 pairs + 3 |p|^2 rows + 3 |g|^2 rows the contraction dim
is K=24, all bf16, accumulated exactly into fp32 PSUM.

Per PSUM megatile [128, 1024] (2 banks): ScalarE copies d2 to SBUF; DVE
takes the free-axis row min (2x mode from SBUF) and the running column-state
min; PE transposes the final column state so DVE can finish the column mins.
sqrt + means run on the host (min/max commute with sqrt/clamp).

Each core handles 4 of the 32 batches. No collectives; host combines scalars.
"""

import sys

for _p in ("/opt/trn_rl_repo",):
    if _p not in sys.path:
        sys.path.insert(0, _p)

from contextlib import ExitStack
from functools import lru_cache

import ml_dtypes
import numpy as np

import concourse.bass as bass
import concourse.tile as tile
from concourse import bacc, masks, mybir
from concourse.bass_utils import run_bass_kernel_spmd

F32 = mybir.dt.float32
BF16 = mybir.dt.bfloat16
MIN = mybir.AluOpType.min
NPBF16 = ml_dtypes.bfloat16

B, N, M = 32, 4096, 4096
NCORES = 8
BPC = B // NCORES  # batches per core
K = 24             # augmented contraction dim (3 coords x 6 bf16 pairs + 2x3 norm rows)
PCH = 128          # pred chunk size (PE partitions)
NP = N // PCH      # 32 pred chunks
FD = 1024          # psum tile free size (2 PSUM banks)
NG = M // FD       # 4 gt chunks
MMN = 512          # matmul moving free dim (one fp32 PSUM bank)
BIG = 3.0e38


def _build_program():
    nc = bacc.Bacc(
        "TRN2", target_bir_lowering=False, debug=False, num_devices=NCORES
    )
    lhs = nc.dram_tensor("lhs", [BPC * K, N], BF16, kind="ExternalInput").ap()
    rhs = nc.dram_tensor("rhs", [BPC * K, M], BF16, kind="ExternalInput").ap()
    rowmin = nc.dram_tensor("rowmin", [BPC * PCH, NP], F32, kind="ExternalOutput").ap()
    colmin = nc.dram_tensor(
        "colmin", [BPC * PCH, M // PCH], F32, kind="ExternalOutput"
    ).ap()

    with tile.TileContext(nc) as tc, ExitStack() as ctx:
        const_pool = ctx.enter_context(tc.tile_pool(name="const", bufs=1))
        ident = const_pool.tile([PCH, PCH], F32)
        masks.make_identity(nc, ident[:])
        inf_t = const_pool.tile([PCH, FD], F32)
        nc.gpsimd.memset(inf_t[:], BIG)

        lr_pool = ctx.enter_context(tc.tile_pool(name="lr", bufs=2))
        col_pool = ctx.enter_context(tc.tile_pool(name="col", bufs=NG + 1))
        d2_pool = ctx.enter_context(tc.tile_pool(name="d2", bufs=4))
        acc_pool = ctx.enter_context(tc.tile_pool(name="acc", bufs=2))
        scr_pool = ctx.enter_context(tc.tile_pool(name="scr", bufs=8))
        psum_pool = ctx.enter_context(tc.tile_pool(name="psum", bufs=3, space="PSUM"))
        psumt_pool = ctx.enter_context(tc.tile_pool(name="psumt", bufs=2, space="PSUM"))

        for i in range(BPC):
            L = lr_pool.tile([K, N], BF16, tag="L")
            nc.sync.dma_start(L[:], lhs[K * i : K * (i + 1), :])
            R = lr_pool.tile([K, M], BF16, tag="R")
            nc.sync.dma_start(R[:], rhs[K * i : K * (i + 1), :])

            colstate = [
                col_pool.tile([PCH, FD], F32, tag="cs", name=f"cs_{i}_{g}")
                for g in range(NG)
            ]
            rowacc = acc_pool.tile([PCH, NP], F32, tag="rowacc")
            colfin = acc_pool.tile([PCH, M // PCH], F32, tag="colfin")

            for p in range(NP):
                rowpart = scr_pool.tile(
                    [PCH, NG], F32, tag="rowpart", name=f"rp_{i}_{p}"
                )
                for g in range(NG):
                    ps = psum_pool.tile([PCH, FD], F32, tag="ps")
                    for s in range(FD // MMN):
                        nc.tensor.matmul(
                            ps[:, MMN * s : MMN * (s + 1)],
                            lhsT=L[:, PCH * p : PCH * (p + 1)],
                            rhs=R[:, FD * g + MMN * s : FD * g + MMN * (s + 1)],
                            start=True,
                            stop=True,
                        )
                    # stage d2 into SBUF on the otherwise-idle ScalarE so both
                    # DVE consumers below run from SBUF (reduce gets 2x mode)
                    d2 = d2_pool.tile([PCH, FD], F32, tag="d2")
                    nc.scalar.copy(d2[:], ps[:])
                    # row (pred-point) partial min over this gt chunk
                    nc.vector.tensor_reduce(
                        out=rowpart[:, g : g + 1],
                        in_=d2[:],
                        axis=mybir.AxisListType.X,
                        op=MIN,
                    )
                    # colstate accumulation (min over pred chunks).
                    src0 = inf_t[:] if p == 0 else colstate[g][:]
                    nc.vector.tensor_tensor(
                        out=colstate[g][:], in0=src0, in1=d2[:], op=MIN
                    )
                # fold the NG partials into the final row min for chunk p
                nc.vector.tensor_reduce(
                    out=rowacc[:, p : p + 1],
                    in_=rowpart[:],
                    axis=mybir.AxisListType.X,
                    op=MIN,
                )

            # Column mins: partition-reduce each colstate tile via PE
            # transpose + free-axis min.
            for g in range(NG):
                for blk in range(FD // PCH):
                    pt = psumt_pool.tile([PCH, PCH], F32, tag="pt")
                    nc.tensor.transpose(
                        pt[:], colstate[g][:, PCH * blk : PCH * (blk + 1)], ident[:]
                    )
                    j = g * (FD // PCH) + blk
                    nc.vector.tensor_reduce(
                        out=colfin[:, j : j + 1],
                        in_=pt[:],
                        axis=mybir.AxisListType.X,
                        op=MIN,
                    )

            nc.sync.dma_start(rowmin[PCH * i : PCH * (i + 1), :], rowacc[:])
            nc.sync.dma_start(colmin[PCH * i : PCH * (i + 1), :], colfin[:])

    nc.compile()
    return nc


@lru_cache(maxsize=1)
def _get_program():
    return _build_program()


def _split3(x):
    """fp32 -> three bf16 terms whose sum matches x to ~2^-27 rel."""
    h = x.astype(NPBF16)
    r = x - h.astype(np.float32)
    m = r.astype(NPBF16)
    l = (r - m.astype(np.float32)).astype(NPBF16)
    return h, m, l


def _make_inputs(pred, gt):
    """Host-side packing of the K=24 bf16 split operands, per core."""
    pred = np.ascontiguousarray(pred, dtype=np.float32)
    gt = np.ascontiguousarray(gt, dtype=np.float32)
    p2 = np.einsum("bnd,bnd->bn", pred, pred)
    g2 = np.einsum("bmd,bmd->bm", gt, gt)
    Lr, Rr = [], []
    for d in range(3):
        u = np.float32(-2.0) * pred[:, :, d]
        v = gt[:, :, d]
        uh, um, ul = _split3(u)
        vh, vm, vl = _split3(v)
        # product pairs kept: hh, hm, mh, hl, lh, mm
        Lr += [uh, uh, um, uh, ul, um]
        Rr += [vh, vm, vh, vl, vh, vm]
    ph, pm, pl = _split3(p2)
    gh, gm, gl = _split3(g2)
    ones_n = np.ones_like(p2, dtype=NPBF16)
    ones_m = np.ones_like(g2, dtype=NPBF16)
    Lr += [ph, pm, pl, ones_n, ones_n, ones_n]
    Rr += [ones_m, ones_m, ones_m, gh, gm, gl]
    lhs = np.stack(Lr, axis=1)  # [B, K, N] bf16
    rhs = np.stack(Rr, axis=1)  # [B, K, M] bf16
    in_maps = []
    for c in range(NCORES):
        sl = slice(c * BPC, (c + 1) * BPC)
        in_maps.append(
            {
                "lhs": np.ascontiguousarray(lhs[sl].reshape(BPC * K, N)),
                "rhs": np.ascontiguousarray(rhs[sl].reshape(BPC * K, M)),
            }
        )
    return in_maps


def _finish(results):
    rowmins = np.stack([r["rowmin"] for r in results])  # [8, BPC*128, 32]
    colmins = np.stack([r["colmin"] for r in results])
    ch2 = np.sqrt(np.maximum(rowmins.astype(np.float64), 1e-12)).mean()
    ch1 = np.sqrt(np.maximum(colmins.astype(np.float64), 1e-12)).mean()
    return np.asarray(ch1 + ch2, dtype=np.float32)


def kernel(pred, gt):
    nc = _get_program()
    in_maps = _make_inputs(pred, gt)
    res = run_bass_kernel_spmd(nc, in_maps, list(range(NCORES)))
    return _finish(res.results)


if __name__ == "__main__":
    rng = np.random.default_rng(0)
    pred = rng.standard_normal((B, N, 3), dtype=np.float32)
    gt = rng.standard_normal((B, M, 3), dtype=np.float32)
    print(kernel(pred, gt))
